# revision 7
# baseline (speedup 1.0000x reference)
"""Trainium2 Bass kernel for nn_AttentionHelper (B=8, C=128, Lq=Lk=2048).

reference:
    energy   = einsum('bcq,bck->bqk', Q, K) * (1/sqrt(C))
    attention= softmax(energy + log(mask + 1e-9), axis=-1) * mask
    out      = einsum('bck,bqk->bcq', V, attention)
returns (out, attention).

Sharding: data-parallel over batch B — one batch per NeuronCore (8 cores).

Per-core algorithm (all f32):
  - K, V, Q resident in SBUF; V pre-transposed via PE (16 128x128 blocks).
  - For each of 16 q-tiles (128 rows):
      e = Q_tile^T K             (PE, PSUM, 4 N=512 matmuls)
      madd = mask*(-K1) + K1     (GPSIMD, int32->f32; K1 = log(1e-9) so
                                  madd = log(mask+1e-9) exactly in fp32)
      t = e*scale + madd         (DVE scalar_tensor_tensor from PSUM)
      p = exp(t), D = rowsum(p)  (ACT activation w/ accum_out - one pass)
      r = 1/D                    (DVE reciprocal)
      A = p * r                  (DVE tensor_scalar, per-partition scalar)
      store A -> attention       (masked entries carry ~1e-9*A instead of
                                  exact 0; abs err ~1e-9, well under fp32
                                  noise of the reference)
      A^T blocks via PE transpose -> PSUM -> ACT copy -> SBUF
  - Per 512-q chunk: out_chunk = sum_kb V^T_kb @ A^T[kb]  (16 accumulating
    matmuls), ACT drain, DMA out.
"""
import numpy as np

B, C, LQ, LK = 8, 128, 2048, 2048
P = 128
NTILES = LQ // P            # 16 q-tiles
NKB = LK // P               # 16 k-blocks
CHUNK = 4                   # q-tiles per PV chunk (512 q columns)

SCALE = 1.0 / float(np.sqrt(np.float64(C)))
K1 = float(np.log(np.float32(1e-9), dtype=np.float32))  # -20.723267


def _split_excess_waits(nc, max_inline=1):
    """This walrus build accepts at most one sync-wait per instruction
    (f32 Matmult keeps LDWEIGHTS fused; STT/Drain structs too). Hoist all
    but one wait onto standalone same-engine EventSemaphore instructions."""
    import concourse.mybir as mybir

    n_split = 0
    for bb in nc.main_func.blocks:
        new_list = []
        changed = False
        for ins in bb.instructions:
            si = ins.sync_info
            if si is not None and si.on_wait and len(si.on_wait) > max_inline:
                waits = list(si.on_wait)
                hoistable = [w for w in waits if w.wait_reg is None]
                inline = [w for w in waits if w.wait_reg is not None]
                while hoistable and len(inline) < max_inline:
                    inline.append(hoistable.pop())
                for w in hoistable:
                    es = mybir.InstEventSemaphore(
                        name=f"I-waitsplit-{nc.next_id()}", ins=[], outs=[]
                    )
                    es.engine = ins.engine
                    es.sync_info = mybir.SyncInfo(
                        on_wait=[
                            mybir.SyncWait(
                                sync_type=w.sync_type,
                                id=w.id,
                                wait_mode=w.wait_mode,
                                ant_name=w.ant_name,
                                wait_value=w.wait_value,
                            )
                        ],
                        on_update=[],
                    )
                    new_list.append(es)
                    n_split += 1
                ins.sync_info = mybir.SyncInfo(
                    on_wait=inline, on_update=list(si.on_update)
                )
                changed = True
            new_list.append(ins)
        if changed:
            bb.instructions = new_list
    return n_split


def build_program():
    import concourse.bass as bass
    import concourse.tile as tile
    from concourse import mybir
    from concourse.masks import make_identity

    f32 = mybir.dt.float32
    f32r = mybir.dt.float32r
    i32 = mybir.dt.int32
    Alu = mybir.AluOpType
    Act = mybir.ActivationFunctionType

    nc = bass.Bass("TRN2", debug=False)

    q_d = nc.dram_tensor("pq", [C, LQ], f32, kind="ExternalInput").ap()
    k_d = nc.dram_tensor("pk", [C, LK], f32, kind="ExternalInput").ap()
    v_d = nc.dram_tensor("pv", [C, LK], f32, kind="ExternalInput").ap()
    m_d = nc.dram_tensor("pm", [LQ, LK], i32, kind="ExternalInput").ap()
    att_d = nc.dram_tensor("att", [LQ, LK], f32, kind="ExternalOutput").ap()
    out_d = nc.dram_tensor("out", [C, LQ], f32, kind="ExternalOutput").ap()

    with tile.TileContext(nc) as tc:
        with (
            tc.tile_pool(name="singles", bufs=1) as singles,
            tc.tile_pool(name="masks", bufs=3) as masks,
            tc.tile_pool(name="madds", bufs=3) as madds,
            tc.tile_pool(name="tsbs", bufs=2) as tsbs,
            tc.tile_pool(name="psbs", bufs=2) as psbs,
            tc.tile_pool(name="asbs", bufs=3) as asbs,
            tc.tile_pool(name="atsbs", bufs=1) as atsbs,
            tc.tile_pool(name="outs", bufs=2) as outs,
            tc.tile_pool(name="smalls", bufs=4) as smalls,
            tc.tile_pool(name="ps_e", bufs=2, space="PSUM") as ps_e,
            tc.tile_pool(name="ps_at", bufs=2, space="PSUM") as ps_at,
            tc.tile_pool(name="ps_pv", bufs=1, space="PSUM") as ps_pv,
        ):
            # ---- setup: resident tensors --------------------------------
            # f32r matmul operands must be produced "rounded": stage the f32
            # DMA then round via compute copy into f32r tiles.
            qsb = singles.tile([P, LQ], f32r)
            ksb = singles.tile([P, LK], f32r)
            vsb = singles.tile([P, LK], f32)
            stage_q = masks.tile([P, LQ], f32, tag="stage")
            nc.gpsimd.dma_start(out=stage_q, in_=q_d)
            nc.vector.tensor_copy(qsb, stage_q)
            stage_k = masks.tile([P, LK], f32, tag="stage")
            nc.gpsimd.dma_start(out=stage_k, in_=k_d)
            nc.vector.tensor_copy(ksb, stage_k)
            nc.gpsimd.dma_start(out=vsb, in_=v_d)

            ident = singles.tile([P, P], f32)
            make_identity(nc, ident)
            ident_r = singles.tile([P, P], f32r)
            nc.vector.tensor_copy(ident_r, ident)

            # V^T blocks: vt[:, kb, :] = V[:, kb*128:(kb+1)*128]^T
            vtsb = singles.tile([P, NKB, P], f32r)
            for g in range(NKB // 4):
                vt_ps = ps_at.tile([P, 4 * P], f32, tag="at_ps")
                for j in range(4):
                    kb = g * 4 + j
                    nc.tensor.transpose(
                        vt_ps[:, j * P:(j + 1) * P],
                        vsb[:, kb * P:(kb + 1) * P],
                        ident,
                    )
                nc.scalar.copy(
                    out=vtsb[:, g * 4:(g + 1) * 4, :],
                    in_=vt_ps.rearrange("p (a b) -> p a b", a=4),
                )

            # ---- main loop ----------------------------------------------
            for chunk in range(NTILES // CHUNK):
                atsb = atsbs.tile([P, NKB, CHUNK * P], f32r)
                for t in range(CHUNK):
                    qt = chunk * CHUNK + t
                    qs = qt * P

                    mask_t = masks.tile([P, LK], i32)
                    nc.sync.dma_start(out=mask_t, in_=m_d[qs:qs + P, :])

                    # madd = log(mask + 1e-9) exactly (0.0 or K1)
                    madd = madds.tile([P, LK], f32)
                    nc.gpsimd.tensor_scalar(
                        out=madd, in0=mask_t, scalar1=-K1, scalar2=K1,
                        op0=Alu.mult, op1=Alu.add,
                    )

                    # e = Q_tile^T @ K ; t = e*scale + madd  (1024-col halves)
                    t_sb = tsbs.tile([P, LK], f32)
                    for h in range(2):
                        e_ps = ps_e.tile([P, 1024], f32, tag="e_ps")
                        for n in range(2):
                            col = h * 1024 + n * 512
                            nc.tensor.matmul(
                                e_ps[:, n * 512:(n + 1) * 512],
                                qsb[:, qs:qs + P],
                                ksb[:, col:col + 512],
                                start=True, stop=True,
                            )
                        nc.vector.scalar_tensor_tensor(
                            out=t_sb[:, h * 1024:(h + 1) * 1024],
                            in0=e_ps, scalar=SCALE,
                            in1=madd[:, h * 1024:(h + 1) * 1024],
                            op0=Alu.mult, op1=Alu.add,
                        )

                    # p = exp(t), D = rowsum(p)
                    p_sb = psbs.tile([P, LK], f32)
                    d_sb = smalls.tile([P, 1], f32, tag="d")
                    nc.scalar.activation(
                        out=p_sb, in_=t_sb, func=Act.Exp,
                        bias=0.0, scale=1.0, accum_out=d_sb,
                    )
                    r_sb = smalls.tile([P, 1], f32, tag="r")
                    nc.vector.reciprocal(out=r_sb, in_=d_sb)

                    # A = p * (1/D)
                    a_sb = asbs.tile([P, LK], f32r)
                    nc.vector.tensor_scalar_mul(a_sb, p_sb, r_sb)
                    nc.scalar.dma_start(out=att_d[qs:qs + P, :], in_=a_sb.bitcast(f32))

                    # A^T blocks -> atsb[:, kb, t*128:(t+1)*128]
                    for g in range(NKB // 4):
                        at_ps = ps_at.tile([P, 4 * P], f32, tag="at_ps")
                        for j in range(4):
                            kb = g * 4 + j
                            nc.tensor.transpose(
                                at_ps[:, j * P:(j + 1) * P].bitcast(f32r),
                                a_sb[:, kb * P:(kb + 1) * P],
                                ident_r,
                            )
                        nc.scalar.copy(
                            out=atsb[:, g * 4:(g + 1) * 4, t * P:(t + 1) * P],
                            in_=at_ps.rearrange("p (a b) -> p a b", a=4),
                        )

                # out chunk: sum over k-blocks of V^T_kb @ A^T[kb]
                pv_ps = ps_pv.tile([P, CHUNK * P], f32)
                for kb in range(NKB):
                    nc.tensor.matmul(
                        pv_ps, vtsb[:, kb, :],
                        atsb[:, kb, :],
                        start=(kb == 0), stop=(kb == NKB - 1),
                    )
                o_sb = outs.tile([P, CHUNK * P], f32)
                nc.scalar.copy(out=o_sb, in_=pv_ps)
                nc.scalar.dma_start(
                    out=out_d[:, chunk * CHUNK * P:(chunk + 1) * CHUNK * P],
                    in_=o_sb,
                )

    _split_excess_waits(nc)
    return nc


_NC = None


def _get_nc():
    global _NC
    if _NC is None:
        _NC = build_program()
    return _NC


def make_in_maps(proj_query, proj_key, proj_val, padding_mask):
    in_maps = []
    for b in range(B):
        in_maps.append({
            "pq": np.ascontiguousarray(proj_query[b], dtype=np.float32),
            "pk": np.ascontiguousarray(proj_key[b], dtype=np.float32),
            "pv": np.ascontiguousarray(proj_val[b], dtype=np.float32),
            "pm": np.ascontiguousarray(padding_mask[b], dtype=np.int32),
        })
    return in_maps


def kernel(proj_query, proj_key, proj_val, padding_mask):
    from concourse.bass_utils import run_bass_kernel_spmd

    nc = _get_nc()
    in_maps = make_in_maps(proj_query, proj_key, proj_val, padding_mask)
    res = run_bass_kernel_spmd(nc, in_maps, core_ids=list(range(B)))
    out = np.stack([res.results[b]["out"] for b in range(B)])
    att = np.stack([res.results[b]["att"] for b in range(B)])
    return out, att


# revision 26
# speedup vs baseline: 1.0398x; 1.0398x over previous
"""Trainium2 Bass kernel for nn_AttentionHelper (B=8, C=128, Lq=Lk=2048).

reference:
    energy   = einsum('bcq,bck->bqk', Q, K) * (1/sqrt(C))
    attention= softmax(energy + log(mask + 1e-9), axis=-1) * mask
    out      = einsum('bck,bqk->bcq', V, attention)
returns (out, attention).

Sharding: data-parallel over batch B — one batch per NeuronCore (8 cores).

Per-core algorithm (all f32):
  - K, V, Q resident in SBUF; V pre-transposed via PE (16 128x128 blocks).
  - For each of 16 q-tiles (128 rows):
      e = Q_tile^T K             (PE, PSUM, 4 N=512 matmuls)
      madd = mask*(-K1) + K1     (GPSIMD, int32->f32; K1 = log(1e-9) so
                                  madd = log(mask+1e-9) exactly in fp32)
      t = e*scale + madd         (DVE scalar_tensor_tensor from PSUM)
      p = exp(t), D = rowsum(p)  (ACT activation w/ accum_out - one pass)
      r = 1/D                    (DVE reciprocal)
      A = p * r                  (DVE tensor_scalar, per-partition scalar)
      store A -> attention       (masked entries carry ~1e-9*A instead of
                                  exact 0; abs err ~1e-9, well under fp32
                                  noise of the reference)
      A^T blocks via PE transpose -> PSUM -> ACT copy -> SBUF
  - Per 512-q chunk: out_chunk = sum_kb V^T_kb @ A^T[kb]  (16 accumulating
    matmuls), ACT drain, DMA out.
"""
import numpy as np

B, C, LQ, LK = 8, 128, 2048, 2048
P = 128
NTILES = LQ // P            # 16 q-tiles
NKB = LK // P               # 16 k-blocks
CHUNK = 2                   # q-tiles per PV chunk (256 q columns)

SCALE = 1.0 / float(np.sqrt(np.float64(C)))
K1 = float(np.log(np.float32(1e-9), dtype=np.float32))  # -20.723267


def _split_excess_waits(nc, max_inline=1):
    """This walrus build accepts at most one sync-wait per instruction
    (f32 Matmult keeps LDWEIGHTS fused; STT/Drain structs too). Hoist all
    but one wait onto standalone same-engine EventSemaphore instructions."""
    import concourse.mybir as mybir

    n_split = 0
    for bb in nc.main_func.blocks:
        new_list = []
        changed = False
        for ins in bb.instructions:
            si = ins.sync_info
            if si is not None and si.on_wait and len(si.on_wait) > max_inline:
                waits = list(si.on_wait)
                hoistable = [w for w in waits if w.wait_reg is None]
                inline = [w for w in waits if w.wait_reg is not None]
                while hoistable and len(inline) < max_inline:
                    inline.append(hoistable.pop())
                for w in hoistable:
                    es = mybir.InstEventSemaphore(
                        name=f"I-waitsplit-{nc.next_id()}", ins=[], outs=[]
                    )
                    es.engine = ins.engine
                    es.sync_info = mybir.SyncInfo(
                        on_wait=[
                            mybir.SyncWait(
                                sync_type=w.sync_type,
                                id=w.id,
                                wait_mode=w.wait_mode,
                                ant_name=w.ant_name,
                                wait_value=w.wait_value,
                            )
                        ],
                        on_update=[],
                    )
                    new_list.append(es)
                    n_split += 1
                ins.sync_info = mybir.SyncInfo(
                    on_wait=inline, on_update=list(si.on_update)
                )
                changed = True
            new_list.append(ins)
        if changed:
            bb.instructions = new_list
    return n_split


def build_program():
    import concourse.bass as bass
    import concourse.tile as tile
    from concourse import mybir
    from concourse.masks import make_identity

    f32 = mybir.dt.float32
    f32r = mybir.dt.float32r
    i32 = mybir.dt.int32
    Alu = mybir.AluOpType
    Act = mybir.ActivationFunctionType

    nc = bass.Bass("TRN2", debug=False)

    q_d = nc.dram_tensor("pq", [C, LQ], f32, kind="ExternalInput").ap()
    k_d = nc.dram_tensor("pk", [C, LK], f32, kind="ExternalInput").ap()
    v_d = nc.dram_tensor("pv", [C, LK], f32, kind="ExternalInput").ap()
    m_d = nc.dram_tensor("pm", [LQ, LK], i32, kind="ExternalInput").ap()
    att_d = nc.dram_tensor("att", [LQ, LK], f32, kind="ExternalOutput").ap()
    out_d = nc.dram_tensor("out", [C, LQ], f32, kind="ExternalOutput").ap()

    with tile.TileContext(nc) as tc:
        with (
            tc.tile_pool(name="singles", bufs=1) as singles,
            tc.tile_pool(name="masks", bufs=3) as masks,
            tc.tile_pool(name="madds", bufs=2) as madds,
            tc.tile_pool(name="tsbs", bufs=2) as tsbs,
            tc.tile_pool(name="psbs", bufs=2) as psbs,
            tc.tile_pool(name="asbs", bufs=2) as asbs,
            tc.tile_pool(name="atsbs", bufs=2) as atsbs,
            tc.tile_pool(name="outs", bufs=2) as outs,
            tc.tile_pool(name="smalls", bufs=4) as smalls,
            tc.tile_pool(name="ps_e", bufs=2, space="PSUM") as ps_e,
            tc.tile_pool(name="ps_at", bufs=3, space="PSUM") as ps_at,
            tc.tile_pool(name="ps_pv", bufs=1, space="PSUM") as ps_pv,
        ):
            # ---- setup: resident tensors --------------------------------
            # f32r matmul operands must be produced "rounded": stage the f32
            # DMA then round via compute copy into f32r tiles.
            qsb = singles.tile([P, LQ], f32r)
            ksb = singles.tile([P, LK], f32r)
            vsb = singles.tile([P, LK], f32)
            stage_k = masks.tile([P, LK // 2], f32, tag="stage")
            nc.sync.dma_start(out=stage_k, in_=k_d[:, 0:LK // 2])
            nc.vector.tensor_copy(ksb[:, 0:LK // 2], stage_k)
            stage_q = masks.tile([P, LQ // 2], f32, tag="stage")
            nc.sync.dma_start(out=stage_q, in_=q_d[:, 0:LQ // 2])
            nc.vector.tensor_copy(qsb[:, 0:LQ // 2], stage_q)
            prefetched_masks = []
            mk = masks.tile([P, LK], i32, tag="mask")
            nc.sync.dma_start(out=mk, in_=m_d[0:P, :])
            prefetched_masks.append(mk)
            stage_k2 = masks.tile([P, LK // 2], f32, tag="stage")
            nc.sync.dma_start(out=stage_k2, in_=k_d[:, LK // 2:])
            nc.vector.tensor_copy(ksb[:, LK // 2:], stage_k2)
            mk1 = masks.tile([P, LK], i32, tag="mask")
            nc.sync.dma_start(out=mk1, in_=m_d[P:2 * P, :])
            prefetched_masks.append(mk1)
            stage_q2 = masks.tile([P, LQ // 2], f32, tag="stage")
            nc.sync.dma_start(out=stage_q2, in_=q_d[:, LQ // 2:])
            nc.vector.tensor_copy(qsb[:, LQ // 2:], stage_q2)
            nc.sync.dma_start(out=vsb, in_=v_d)

            ident = singles.tile([P, P], f32)
            make_identity(nc, ident)
            ident_r = singles.tile([P, P], f32r)
            nc.vector.tensor_copy(ident_r, ident)

            # V^T blocks: vt[:, kb, :] = V[:, kb*128:(kb+1)*128]^T
            vtsb = singles.tile([P, NKB, P], f32r)
            for g in range(NKB // 4):
                vt_ps = ps_at.tile([P, 4 * P], f32, tag="at_ps")
                for j in range(4):
                    kb = g * 4 + j
                    nc.tensor.transpose(
                        vt_ps[:, j * P:(j + 1) * P],
                        vsb[:, kb * P:(kb + 1) * P],
                        ident,
                    )
                nc.scalar.copy(
                    out=vtsb[:, g * 4:(g + 1) * 4, :],
                    in_=vt_ps.rearrange("p (a b) -> p a b", a=4),
                )

            # ---- main loop ----------------------------------------------
            for chunk in range(NTILES // CHUNK):
                atsb = atsbs.tile([P, NKB, CHUNK * P], f32r)
                for t in range(CHUNK):
                    qt = chunk * CHUNK + t
                    qs = qt * P

                    if qt < 2:
                        mask_t = prefetched_masks[qt]
                    else:
                        mask_t = masks.tile([P, LK], i32, tag="mask")
                        nc.sync.dma_start(out=mask_t, in_=m_d[qs:qs + P, :])

                    # madd = log(mask + 1e-9) exactly (0.0 or K1)
                    madd = madds.tile([P, LK], f32)
                    nc.gpsimd.tensor_scalar(
                        out=madd, in0=mask_t, scalar1=-K1, scalar2=K1,
                        op0=Alu.mult, op1=Alu.add,
                    )

                    # e = Q_tile^T @ K ; t = e*scale + madd  (1024-col halves)
                    t_sb = tsbs.tile([P, LK], f32)
                    for h in range(2):
                        e_ps = ps_e.tile([P, 1024], f32, tag="e_ps")
                        for n in range(2):
                            col = h * 1024 + n * 512
                            nc.tensor.matmul(
                                e_ps[:, n * 512:(n + 1) * 512],
                                qsb[:, qs:qs + P],
                                ksb[:, col:col + 512],
                                start=True, stop=True,
                            )
                        nc.vector.scalar_tensor_tensor(
                            out=t_sb[:, h * 1024:(h + 1) * 1024],
                            in0=e_ps, scalar=SCALE,
                            in1=madd[:, h * 1024:(h + 1) * 1024],
                            op0=Alu.mult, op1=Alu.add,
                        )

                    # p = exp(t), D = rowsum(p)
                    p_sb = psbs.tile([P, LK], f32)
                    d_sb = smalls.tile([P, 1], f32, tag="d")
                    nc.scalar.activation(
                        out=p_sb, in_=t_sb, func=Act.Exp,
                        bias=0.0, scale=1.0, accum_out=d_sb,
                    )
                    r_sb = smalls.tile([P, 1], f32, tag="r")
                    nc.vector.reciprocal(out=r_sb, in_=d_sb)

                    # A = p * (1/D)
                    a_sb = asbs.tile([P, LK], f32r)
                    nc.vector.tensor_scalar_mul(a_sb, p_sb, r_sb)
                    (nc.scalar if qt % 2 == 0 else nc.sync).dma_start(out=att_d[qs:qs + P, :], in_=a_sb.bitcast(f32))

                    # A^T blocks -> atsb[:, kb, t*128:(t+1)*128]
                    for g in range(NKB // 4):
                        at_ps = ps_at.tile([P, 4 * P], f32, tag="at_ps")
                        for j in range(4):
                            kb = g * 4 + j
                            nc.tensor.transpose(
                                at_ps[:, j * P:(j + 1) * P].bitcast(f32r),
                                a_sb[:, kb * P:(kb + 1) * P],
                                ident_r,
                            )
                        if g == 3:
                            nc.vector.tensor_copy(
                                atsb[:, g * 4:(g + 1) * 4, t * P:(t + 1) * P],
                                at_ps.rearrange("p (a b) -> p a b", a=4),
                            )
                        else:
                            nc.scalar.copy(
                                out=atsb[:, g * 4:(g + 1) * 4, t * P:(t + 1) * P],
                                in_=at_ps.rearrange("p (a b) -> p a b", a=4),
                            )

                # out chunk: sum over k-blocks of V^T_kb @ A^T[kb]
                pv_ps = ps_pv.tile([P, CHUNK * P], f32)
                for kb in range(NKB):
                    nc.tensor.matmul(
                        pv_ps, vtsb[:, kb, :],
                        atsb[:, kb, :],
                        start=(kb == 0), stop=(kb == NKB - 1),
                    )
                o_sb = outs.tile([P, CHUNK * P], f32)
                nc.scalar.copy(out=o_sb, in_=pv_ps)
                nc.scalar.dma_start(
                    out=out_d[:, chunk * CHUNK * P:(chunk + 1) * CHUNK * P],
                    in_=o_sb,
                )

    _split_excess_waits(nc)
    return nc


_NC = None


def _get_nc():
    global _NC
    if _NC is None:
        _NC = build_program()
    return _NC


def make_in_maps(proj_query, proj_key, proj_val, padding_mask):
    in_maps = []
    for b in range(B):
        in_maps.append({
            "pq": np.ascontiguousarray(proj_query[b], dtype=np.float32),
            "pk": np.ascontiguousarray(proj_key[b], dtype=np.float32),
            "pv": np.ascontiguousarray(proj_val[b], dtype=np.float32),
            "pm": np.ascontiguousarray(padding_mask[b], dtype=np.int32),
        })
    return in_maps


def kernel(proj_query, proj_key, proj_val, padding_mask):
    from concourse.bass_utils import run_bass_kernel_spmd

    nc = _get_nc()
    in_maps = make_in_maps(proj_query, proj_key, proj_val, padding_mask)
    res = run_bass_kernel_spmd(nc, in_maps, core_ids=list(range(B)))
    out = np.stack([res.results[b]["out"] for b in range(B)])
    att = np.stack([res.results[b]["att"] for b in range(B)])
    return out, att


# revision 32
# speedup vs baseline: 1.1989x; 1.1530x over previous
"""Trainium2 Bass kernel for nn_AttentionHelper (B=8, C=128, Lq=Lk=2048).

reference:
    energy   = einsum('bcq,bck->bqk', Q, K) * (1/sqrt(C))
    attention= softmax(energy + log(mask + 1e-9), axis=-1) * mask
    out      = einsum('bck,bqk->bcq', V, attention)
returns (out, attention).

Sharding: data-parallel over batch B — one batch per NeuronCore (8 cores).

Per-core algorithm (all f32):
  - K, V, Q resident in SBUF; V pre-transposed via PE (16 128x128 blocks).
  - For each of 16 q-tiles (128 rows):
      e = Q_tile^T K             (PE, PSUM, 4 N=512 matmuls)
      madd = mask*(-K1) + K1     (GPSIMD, int32->f32; K1 = log(1e-9) so
                                  madd = log(mask+1e-9) exactly in fp32)
      t = e*scale + madd         (DVE scalar_tensor_tensor from PSUM)
      p = exp(t), D = rowsum(p)  (ACT activation w/ accum_out - one pass)
      r = 1/D                    (DVE reciprocal)
      A = p * r                  (DVE tensor_scalar, per-partition scalar)
      store A -> attention       (masked entries carry ~1e-9*A instead of
                                  exact 0; abs err ~1e-9, well under fp32
                                  noise of the reference)
      A^T blocks via PE transpose -> PSUM -> ACT copy -> SBUF
  - Per 512-q chunk: out_chunk = sum_kb V^T_kb @ A^T[kb]  (16 accumulating
    matmuls), ACT drain, DMA out.
"""
import numpy as np

B, C, LQ, LK = 8, 128, 2048, 2048
P = 128
NTILES = LQ // P            # 16 q-tiles
NKB = LK // P               # 16 k-blocks
CHUNK = 2                   # q-tiles per PV chunk (256 q columns)

SCALE = 1.0 / float(np.sqrt(np.float64(C)))
K1 = float(np.log(np.float32(1e-9), dtype=np.float32))  # -20.723267


def _split_excess_waits(nc, max_inline=1):
    """This walrus build accepts at most one sync-wait per instruction
    (f32 Matmult keeps LDWEIGHTS fused; STT/Drain structs too). Hoist all
    but one wait onto standalone same-engine EventSemaphore instructions."""
    import concourse.mybir as mybir

    n_split = 0
    for bb in nc.main_func.blocks:
        new_list = []
        changed = False
        for ins in bb.instructions:
            si = ins.sync_info
            if si is not None and si.on_wait and len(si.on_wait) > max_inline:
                waits = list(si.on_wait)
                hoistable = [w for w in waits if w.wait_reg is None]
                inline = [w for w in waits if w.wait_reg is not None]
                while hoistable and len(inline) < max_inline:
                    inline.append(hoistable.pop())
                for w in hoistable:
                    es = mybir.InstEventSemaphore(
                        name=f"I-waitsplit-{nc.next_id()}", ins=[], outs=[]
                    )
                    es.engine = ins.engine
                    es.sync_info = mybir.SyncInfo(
                        on_wait=[
                            mybir.SyncWait(
                                sync_type=w.sync_type,
                                id=w.id,
                                wait_mode=w.wait_mode,
                                ant_name=w.ant_name,
                                wait_value=w.wait_value,
                            )
                        ],
                        on_update=[],
                    )
                    new_list.append(es)
                    n_split += 1
                ins.sync_info = mybir.SyncInfo(
                    on_wait=inline, on_update=list(si.on_update)
                )
                changed = True
            new_list.append(ins)
        if changed:
            bb.instructions = new_list
    return n_split


def build_program():
    import concourse.bass as bass
    import concourse.tile as tile
    from concourse import mybir
    from concourse.masks import make_identity

    f32 = mybir.dt.float32
    f32r = mybir.dt.float32r
    i32 = mybir.dt.int32
    Alu = mybir.AluOpType
    Act = mybir.ActivationFunctionType

    nc = bass.Bass("TRN2", debug=False)

    q_d = nc.dram_tensor("pq", [C, LQ], f32, kind="ExternalInput").ap()
    k_d = nc.dram_tensor("pk", [C, LK], f32, kind="ExternalInput").ap()
    v_d = nc.dram_tensor("pv", [C, LK], f32, kind="ExternalInput").ap()
    m_d = nc.dram_tensor("pm", [LQ, LK], i32, kind="ExternalInput").ap()
    att_d = nc.dram_tensor("att", [LQ, LK], f32, kind="ExternalOutput").ap()
    out_d = nc.dram_tensor("out", [C, LQ], f32, kind="ExternalOutput").ap()

    with tile.TileContext(nc) as tc:
        with (
            tc.tile_pool(name="singles", bufs=1) as singles,
            tc.tile_pool(name="masks", bufs=3) as masks,
            tc.tile_pool(name="madds", bufs=3) as madds,
            tc.tile_pool(name="tsbs", bufs=3) as tsbs,
            tc.tile_pool(name="psbs", bufs=2) as psbs,
            tc.tile_pool(name="asbs", bufs=3) as asbs,
            tc.tile_pool(name="atsbs", bufs=2) as atsbs,
            tc.tile_pool(name="outs", bufs=2) as outs,
            tc.tile_pool(name="smalls", bufs=4) as smalls,
            tc.tile_pool(name="ps_e", bufs=2, space="PSUM") as ps_e,
            tc.tile_pool(name="ps_at", bufs=3, space="PSUM") as ps_at,
            tc.tile_pool(name="ps_pv", bufs=1, space="PSUM") as ps_pv,
        ):
            # ---- setup: resident tensors --------------------------------
            # f32r matmul operands must be produced "rounded": stage the f32
            # DMA then round via compute copy into f32r tiles.
            qsb = singles.tile([P, LQ], f32r)
            ksb = singles.tile([P, LK], f32r)
            vsb = singles.tile([P, LK], f32)
            stage_k = masks.tile([P, LK // 2], f32, tag="stage")
            nc.sync.dma_start(out=stage_k, in_=k_d[:, 0:LK // 2])
            nc.vector.tensor_copy(ksb[:, 0:LK // 2], stage_k)
            stage_q = masks.tile([P, LQ // 2], f32, tag="stage")
            nc.sync.dma_start(out=stage_q, in_=q_d[:, 0:LQ // 2])
            nc.vector.tensor_copy(qsb[:, 0:LQ // 2], stage_q)
            prefetched_masks = []
            mk = masks.tile([P, LK], i32, tag="mask")
            nc.sync.dma_start(out=mk, in_=m_d[0:P, :])
            prefetched_masks.append(mk)
            stage_k2 = masks.tile([P, LK // 2], f32, tag="stage")
            nc.sync.dma_start(out=stage_k2, in_=k_d[:, LK // 2:])
            nc.vector.tensor_copy(ksb[:, LK // 2:], stage_k2)
            mk1 = masks.tile([P, LK], i32, tag="mask")
            nc.sync.dma_start(out=mk1, in_=m_d[P:2 * P, :])
            prefetched_masks.append(mk1)
            stage_q2 = masks.tile([P, LQ // 2], f32, tag="stage")
            nc.sync.dma_start(out=stage_q2, in_=q_d[:, LQ // 2:])
            nc.vector.tensor_copy(qsb[:, LQ // 2:], stage_q2)
            nc.sync.dma_start(out=vsb, in_=v_d)

            ident = singles.tile([P, P], f32)
            make_identity(nc, ident)
            ident_r = singles.tile([P, P], f32r)
            nc.vector.tensor_copy(ident_r, ident)

            # V^T blocks: vt[:, kb, :] = V[:, kb*128:(kb+1)*128]^T
            vtsb = singles.tile([P, NKB, P], f32r)
            for g in range(NKB // 4):
                vt_ps = ps_at.tile([P, 4 * P], f32, tag="at_ps")
                for j in range(4):
                    kb = g * 4 + j
                    nc.tensor.transpose(
                        vt_ps[:, j * P:(j + 1) * P],
                        vsb[:, kb * P:(kb + 1) * P],
                        ident,
                    )
                nc.scalar.copy(
                    out=vtsb[:, g * 4:(g + 1) * 4, :],
                    in_=vt_ps.rearrange("p (a b) -> p a b", a=4),
                )

            # ---- main loop ----------------------------------------------
            for chunk in range(NTILES // CHUNK):
                atsb = atsbs.tile([P, NKB, CHUNK * P], f32r)
                for t in range(CHUNK):
                    qt = chunk * CHUNK + t
                    qs = qt * P

                    if qt < 2:
                        mask_t = prefetched_masks[qt]
                    else:
                        mask_t = masks.tile([P, LK], i32, tag="mask")
                        nc.sync.dma_start(out=mask_t, in_=m_d[qs:qs + P, :])

                    # madd = log(mask + 1e-9) exactly (0.0 or K1)
                    madd = madds.tile([P, LK], f32)
                    nc.gpsimd.tensor_scalar(
                        out=madd, in0=mask_t, scalar1=-K1, scalar2=K1,
                        op0=Alu.mult, op1=Alu.add,
                    )

                    # e = Q_tile^T @ K ; t = e*scale + madd  (1024-col halves)
                    t_sb = tsbs.tile([P, LK], f32)
                    for h in range(2):
                        e_ps = ps_e.tile([P, 1024], f32, tag="e_ps")
                        for n in range(2):
                            col = h * 1024 + n * 512
                            nc.tensor.matmul(
                                e_ps[:, n * 512:(n + 1) * 512],
                                qsb[:, qs:qs + P],
                                ksb[:, col:col + 512],
                                start=True, stop=True,
                            )
                        nc.vector.scalar_tensor_tensor(
                            out=t_sb[:, h * 1024:(h + 1) * 1024],
                            in0=e_ps, scalar=SCALE,
                            in1=madd[:, h * 1024:(h + 1) * 1024],
                            op0=Alu.mult, op1=Alu.add,
                        )

                    # p = exp(t), D = rowsum(p)
                    p_sb = psbs.tile([P, LK], f32)
                    d_sb = smalls.tile([P, 1], f32, tag="d")
                    nc.scalar.activation(
                        out=p_sb, in_=t_sb, func=Act.Exp,
                        bias=0.0, scale=1.0, accum_out=d_sb,
                    )
                    r_sb = smalls.tile([P, 1], f32, tag="r")
                    nc.vector.reciprocal(out=r_sb, in_=d_sb)

                    # A = p * (1/D)
                    a_sb = asbs.tile([P, LK], f32r)
                    nc.vector.tensor_scalar_mul(a_sb, p_sb, r_sb)
                    (nc.scalar if qt % 2 == 0 else nc.sync).dma_start(out=att_d[qs:qs + P, :], in_=a_sb.bitcast(f32))

                    # A^T blocks -> atsb[:, kb, t*128:(t+1)*128]
                    for g in range(NKB // 4):
                        at_ps = ps_at.tile([P, 4 * P], f32, tag="at_ps")
                        for j in range(4):
                            kb = g * 4 + j
                            nc.tensor.transpose(
                                at_ps[:, j * P:(j + 1) * P].bitcast(f32r),
                                a_sb[:, kb * P:(kb + 1) * P],
                                ident_r,
                            )
                        if g == 3:
                            nc.vector.tensor_copy(
                                atsb[:, g * 4:(g + 1) * 4, t * P:(t + 1) * P],
                                at_ps.rearrange("p (a b) -> p a b", a=4),
                            )
                        else:
                            nc.scalar.copy(
                                out=atsb[:, g * 4:(g + 1) * 4, t * P:(t + 1) * P],
                                in_=at_ps.rearrange("p (a b) -> p a b", a=4),
                            )

                # out chunk: sum over k-blocks of V^T_kb @ A^T[kb]
                pv_ps = ps_pv.tile([P, CHUNK * P], f32)
                for kb in range(NKB):
                    nc.tensor.matmul(
                        pv_ps, vtsb[:, kb, :],
                        atsb[:, kb, :],
                        start=(kb == 0), stop=(kb == NKB - 1),
                    )
                o_sb = outs.tile([P, CHUNK * P], f32)
                nc.scalar.copy(out=o_sb, in_=pv_ps)
                nc.scalar.dma_start(
                    out=out_d[:, chunk * CHUNK * P:(chunk + 1) * CHUNK * P],
                    in_=o_sb,
                )

    _split_excess_waits(nc)
    return nc


_NC = None


def _get_nc():
    global _NC
    if _NC is None:
        _NC = build_program()
    return _NC


def make_in_maps(proj_query, proj_key, proj_val, padding_mask):
    in_maps = []
    for b in range(B):
        in_maps.append({
            "pq": np.ascontiguousarray(proj_query[b], dtype=np.float32),
            "pk": np.ascontiguousarray(proj_key[b], dtype=np.float32),
            "pv": np.ascontiguousarray(proj_val[b], dtype=np.float32),
            "pm": np.ascontiguousarray(padding_mask[b], dtype=np.int32),
        })
    return in_maps


def kernel(proj_query, proj_key, proj_val, padding_mask):
    from concourse.bass_utils import run_bass_kernel_spmd

    nc = _get_nc()
    in_maps = make_in_maps(proj_query, proj_key, proj_val, padding_mask)
    res = run_bass_kernel_spmd(nc, in_maps, core_ids=list(range(B)))
    out = np.stack([res.results[b]["out"] for b in range(B)])
    att = np.stack([res.results[b]["att"] for b in range(B)])
    return out, att


# revision 38
# speedup vs baseline: 1.2045x; 1.0047x over previous
"""Trainium2 Bass kernel for nn_AttentionHelper (B=8, C=128, Lq=Lk=2048).

reference:
    energy   = einsum('bcq,bck->bqk', Q, K) * (1/sqrt(C))
    attention= softmax(energy + log(mask + 1e-9), axis=-1) * mask
    out      = einsum('bck,bqk->bcq', V, attention)
returns (out, attention).

Sharding: data-parallel over batch B — one batch per NeuronCore (8 cores).

Per-core algorithm (f32 datapath; matmuls in float32r, which runs at 1
cycle/row vs 4 for float32 and costs ~3e-4 relative rounding on Q/K/A):
  - K, V, Q resident in SBUF (Q/K rounded to f32r); V^T via PE transposes.
  - For each of 16 q-tiles (128 rows):
      e = Q_tile^T K             (PE f32r, PSUM, 4 N=512 matmuls)
      madd = mask*(-K1) + K1     (GPSIMD, int32->f32; K1 = log(1e-9) so
                                  madd = log(mask+1e-9) exactly in fp32)
      t = e*scale + madd         (DVE scalar_tensor_tensor from PSUM)
      p = exp(t), D = rowsum(p)  (ACT activation w/ accum_out - one pass)
      r = 1/D                    (DVE reciprocal)
      A = p * r                  (DVE tensor_scalar, f32r out)
      store A -> attention       (masked entries carry ~1e-9*A instead of
                                  exact 0; abs err ~1e-9, well under the
                                  check threshold)
      A^T blocks via PE f32r transposes -> PSUM -> ACT/DVE copy -> SBUF
  - Per CHUNK q-tiles: out_chunk = sum_kb V^T_kb @ A^T[kb]  (16
    accumulating f32r matmuls), ACT drain, DMA out.
"""
import numpy as np

B, C, LQ, LK = 8, 128, 2048, 2048
P = 128
NTILES = LQ // P            # 16 q-tiles
NKB = LK // P               # 16 k-blocks
CHUNK = 2                   # q-tiles per PV chunk (256 q columns)

SCALE = 1.0 / float(np.sqrt(np.float64(C)))
K1 = float(np.log(np.float32(1e-9), dtype=np.float32))  # -20.723267


def _split_excess_waits(nc, max_inline=1):
    """This walrus build accepts at most one sync-wait per instruction
    (f32 Matmult keeps LDWEIGHTS fused; STT/Drain structs too). Hoist all
    but one wait onto standalone same-engine EventSemaphore instructions."""
    import concourse.mybir as mybir

    n_split = 0
    for bb in nc.main_func.blocks:
        new_list = []
        changed = False
        for ins in bb.instructions:
            si = ins.sync_info
            if si is not None and si.on_wait and len(si.on_wait) > max_inline:
                waits = list(si.on_wait)
                hoistable = [w for w in waits if w.wait_reg is None]
                inline = [w for w in waits if w.wait_reg is not None]
                while hoistable and len(inline) < max_inline:
                    inline.append(hoistable.pop())
                for w in hoistable:
                    es = mybir.InstEventSemaphore(
                        name=f"I-waitsplit-{nc.next_id()}", ins=[], outs=[]
                    )
                    es.engine = ins.engine
                    es.sync_info = mybir.SyncInfo(
                        on_wait=[
                            mybir.SyncWait(
                                sync_type=w.sync_type,
                                id=w.id,
                                wait_mode=w.wait_mode,
                                ant_name=w.ant_name,
                                wait_value=w.wait_value,
                            )
                        ],
                        on_update=[],
                    )
                    new_list.append(es)
                    n_split += 1
                ins.sync_info = mybir.SyncInfo(
                    on_wait=inline, on_update=list(si.on_update)
                )
                changed = True
            new_list.append(ins)
        if changed:
            bb.instructions = new_list
    return n_split


def build_program():
    import concourse.bass as bass
    import concourse.tile as tile
    from concourse import mybir
    from concourse.masks import make_identity

    f32 = mybir.dt.float32
    f32r = mybir.dt.float32r
    i32 = mybir.dt.int32
    Alu = mybir.AluOpType
    Act = mybir.ActivationFunctionType

    nc = bass.Bass("TRN2", debug=False)

    q_d = nc.dram_tensor("pq", [C, LQ], f32, kind="ExternalInput").ap()
    k_d = nc.dram_tensor("pk", [C, LK], f32, kind="ExternalInput").ap()
    v_d = nc.dram_tensor("pv", [C, LK], f32, kind="ExternalInput").ap()
    m_d = nc.dram_tensor("pm", [LQ, LK], i32, kind="ExternalInput").ap()
    att_d = nc.dram_tensor("att", [LQ, LK], f32, kind="ExternalOutput").ap()
    out_d = nc.dram_tensor("out", [C, LQ], f32, kind="ExternalOutput").ap()

    with tile.TileContext(nc) as tc:
        with (
            tc.tile_pool(name="singles", bufs=1) as singles,
            tc.tile_pool(name="masks", bufs=3) as masks,
            tc.tile_pool(name="madds", bufs=3) as madds,
            tc.tile_pool(name="tsbs", bufs=3) as tsbs,
            tc.tile_pool(name="psbs", bufs=2) as psbs,
            tc.tile_pool(name="asbs", bufs=3) as asbs,
            tc.tile_pool(name="atsbs", bufs=2) as atsbs,
            tc.tile_pool(name="outs", bufs=2) as outs,
            tc.tile_pool(name="smalls", bufs=8) as smalls,
            tc.tile_pool(name="ps_e", bufs=2, space="PSUM") as ps_e,
            tc.tile_pool(name="ps_at", bufs=3, space="PSUM") as ps_at,
            tc.tile_pool(name="ps_pv", bufs=1, space="PSUM") as ps_pv,
        ):
            # ---- setup: resident tensors --------------------------------
            # f32r matmul operands must be produced "rounded": stage the f32
            # DMA then round via compute copy into f32r tiles.
            qsb = singles.tile([P, LQ], f32r)
            ksb = singles.tile([P, LK], f32r)
            vsb = singles.tile([P, LK], f32)
            stage_k = masks.tile([P, LK // 2], f32, tag="stage")
            nc.sync.dma_start(out=stage_k, in_=k_d[:, 0:LK // 2])
            nc.vector.tensor_copy(ksb[:, 0:LK // 2], stage_k)
            stage_q = masks.tile([P, LQ // 2], f32, tag="stage")
            nc.sync.dma_start(out=stage_q, in_=q_d[:, 0:LQ // 2])
            nc.vector.tensor_copy(qsb[:, 0:LQ // 2], stage_q)
            prefetched_masks = []
            mk = masks.tile([P, LK], i32, tag="mask")
            nc.sync.dma_start(out=mk, in_=m_d[0:P, :])
            prefetched_masks.append(mk)
            stage_k2 = masks.tile([P, LK // 2], f32, tag="stage")
            nc.sync.dma_start(out=stage_k2, in_=k_d[:, LK // 2:])
            nc.vector.tensor_copy(ksb[:, LK // 2:], stage_k2)
            mk1 = masks.tile([P, LK], i32, tag="mask")
            nc.sync.dma_start(out=mk1, in_=m_d[P:2 * P, :])
            prefetched_masks.append(mk1)
            stage_q2 = masks.tile([P, LQ // 2], f32, tag="stage")
            nc.sync.dma_start(out=stage_q2, in_=q_d[:, LQ // 2:])
            nc.vector.tensor_copy(qsb[:, LQ // 2:], stage_q2)
            nc.sync.dma_start(out=vsb, in_=v_d)

            ident = singles.tile([P, P], f32)
            make_identity(nc, ident)
            ident_r = singles.tile([P, P], f32r)
            nc.vector.tensor_copy(ident_r, ident)

            # V^T blocks: vt[:, kb, :] = V[:, kb*128:(kb+1)*128]^T
            vtsb = singles.tile([P, NKB, P], f32r)
            for g in range(NKB // 4):
                vt_ps = ps_at.tile([P, 4 * P], f32, tag="at_ps")
                for j in range(4):
                    kb = g * 4 + j
                    nc.tensor.transpose(
                        vt_ps[:, j * P:(j + 1) * P],
                        vsb[:, kb * P:(kb + 1) * P],
                        ident,
                    )
                nc.scalar.copy(
                    out=vtsb[:, g * 4:(g + 1) * 4, :],
                    in_=vt_ps.rearrange("p (a b) -> p a b", a=4),
                )

            # ---- main loop ----------------------------------------------
            for chunk in range(NTILES // CHUNK):
                atsb = atsbs.tile([P, NKB, CHUNK * P], f32r)
                for t in range(CHUNK):
                    qt = chunk * CHUNK + t
                    qs = qt * P

                    if qt < 2:
                        mask_t = prefetched_masks[qt]
                    else:
                        mask_t = masks.tile([P, LK], i32, tag="mask")
                        nc.sync.dma_start(out=mask_t, in_=m_d[qs:qs + P, :])

                    # madd = log(mask + 1e-9) exactly (0.0 or K1)
                    madd = madds.tile([P, LK], f32)
                    nc.gpsimd.tensor_scalar(
                        out=madd, in0=mask_t, scalar1=-K1, scalar2=K1,
                        op0=Alu.mult, op1=Alu.add,
                    )

                    # e = Q_tile^T @ K ; t = e*scale + madd  (1024-col halves)
                    t_sb = tsbs.tile([P, LK], f32)
                    for h in range(2):
                        e_ps = ps_e.tile([P, 1024], f32, tag="e_ps")
                        for n in range(2):
                            col = h * 1024 + n * 512
                            nc.tensor.matmul(
                                e_ps[:, n * 512:(n + 1) * 512],
                                qsb[:, qs:qs + P],
                                ksb[:, col:col + 512],
                                start=True, stop=True,
                            )
                        nc.vector.scalar_tensor_tensor(
                            out=t_sb[:, h * 1024:(h + 1) * 1024],
                            in0=e_ps, scalar=SCALE,
                            in1=madd[:, h * 1024:(h + 1) * 1024],
                            op0=Alu.mult, op1=Alu.add,
                        )

                    # p = exp(t), D = rowsum(p)
                    p_sb = psbs.tile([P, LK], f32)
                    d_sb = smalls.tile([P, 1], f32, tag="d")
                    nc.scalar.activation(
                        out=p_sb, in_=t_sb, func=Act.Exp,
                        bias=0.0, scale=1.0, accum_out=d_sb,
                    )
                    r_sb = smalls.tile([P, 1], f32, tag="r")
                    nc.vector.reciprocal(out=r_sb, in_=d_sb)

                    # A = p * (1/D)
                    a_sb = asbs.tile([P, LK], f32r)
                    nc.vector.tensor_scalar_mul(a_sb, p_sb, r_sb)
                    (nc.scalar if qt % 2 == 0 else nc.sync).dma_start(out=att_d[qs:qs + P, :], in_=a_sb.bitcast(f32))

                    # A^T blocks -> atsb[:, kb, t*128:(t+1)*128]
                    for g in range(NKB // 4):
                        at_ps = ps_at.tile([P, 4 * P], f32, tag="at_ps")
                        for j in range(4):
                            kb = g * 4 + j
                            nc.tensor.transpose(
                                at_ps[:, j * P:(j + 1) * P].bitcast(f32r),
                                a_sb[:, kb * P:(kb + 1) * P],
                                ident_r,
                            )
                        if g == 3:
                            nc.vector.tensor_copy(
                                atsb[:, g * 4:(g + 1) * 4, t * P:(t + 1) * P],
                                at_ps.rearrange("p (a b) -> p a b", a=4),
                            )
                        else:
                            nc.scalar.copy(
                                out=atsb[:, g * 4:(g + 1) * 4, t * P:(t + 1) * P],
                                in_=at_ps.rearrange("p (a b) -> p a b", a=4),
                            )

                # out chunk: sum over k-blocks of V^T_kb @ A^T[kb]
                pv_ps = ps_pv.tile([P, CHUNK * P], f32)
                for kb in range(NKB):
                    nc.tensor.matmul(
                        pv_ps, vtsb[:, kb, :],
                        atsb[:, kb, :],
                        start=(kb == 0), stop=(kb == NKB - 1),
                    )
                o_sb = outs.tile([P, CHUNK * P], f32)
                nc.scalar.copy(out=o_sb, in_=pv_ps)
                nc.scalar.dma_start(
                    out=out_d[:, chunk * CHUNK * P:(chunk + 1) * CHUNK * P],
                    in_=o_sb,
                )

    _split_excess_waits(nc)
    return nc


_NC = None


def _get_nc():
    global _NC
    if _NC is None:
        _NC = build_program()
    return _NC


def make_in_maps(proj_query, proj_key, proj_val, padding_mask):
    in_maps = []
    for b in range(B):
        in_maps.append({
            "pq": np.ascontiguousarray(proj_query[b], dtype=np.float32),
            "pk": np.ascontiguousarray(proj_key[b], dtype=np.float32),
            "pv": np.ascontiguousarray(proj_val[b], dtype=np.float32),
            "pm": np.ascontiguousarray(padding_mask[b], dtype=np.int32),
        })
    return in_maps


def kernel(proj_query, proj_key, proj_val, padding_mask):
    from concourse.bass_utils import run_bass_kernel_spmd

    nc = _get_nc()
    in_maps = make_in_maps(proj_query, proj_key, proj_val, padding_mask)
    res = run_bass_kernel_spmd(nc, in_maps, core_ids=list(range(B)))
    out = np.stack([res.results[b]["out"] for b in range(B)])
    att = np.stack([res.results[b]["att"] for b in range(B)])
    return out, att


# revision 41
# speedup vs baseline: 1.2506x; 1.0383x over previous
"""Trainium2 Bass kernel for nn_AttentionHelper (B=8, C=128, Lq=Lk=2048).

reference:
    energy   = einsum('bcq,bck->bqk', Q, K) * (1/sqrt(C))
    attention= softmax(energy + log(mask + 1e-9), axis=-1) * mask
    out      = einsum('bck,bqk->bcq', V, attention)
returns (out, attention).

Sharding: data-parallel over batch B — one batch per NeuronCore (8 cores).

Per-core algorithm (f32 datapath; matmuls in float32r, which runs at 1
cycle/row vs 4 for float32 and costs ~3e-4 relative rounding on Q/K/A):
  - K, V, Q resident in SBUF (Q/K rounded to f32r); V^T via PE transposes.
  - For each of 16 q-tiles (128 rows):
      e = Q_tile^T K             (PE f32r, PSUM, 4 N=512 matmuls)
      madd = mask*(-K1) + K1     (GPSIMD, int32->f32; K1 = log(1e-9) so
                                  madd = log(mask+1e-9) exactly in fp32)
      t = e*scale + madd         (DVE scalar_tensor_tensor from PSUM)
      p = exp(t), D = rowsum(p)  (ACT activation w/ accum_out - one pass)
      r = 1/D                    (DVE reciprocal)
      A = p * r                  (DVE tensor_scalar, f32r out)
      store A -> attention       (masked entries carry ~1e-9*A instead of
                                  exact 0; abs err ~1e-9, well under the
                                  check threshold)
      A^T blocks via PE f32r transposes -> PSUM -> ACT/DVE copy -> SBUF
  - Per CHUNK q-tiles: out_chunk = sum_kb V^T_kb @ A^T[kb]  (16
    accumulating f32r matmuls), ACT drain, DMA out.
"""
import numpy as np

B, C, LQ, LK = 8, 128, 2048, 2048
P = 128
NTILES = LQ // P            # 16 q-tiles
NKB = LK // P               # 16 k-blocks
CHUNK = 2                   # q-tiles per PV chunk (256 q columns)

SCALE = 1.0 / float(np.sqrt(np.float64(C)))
K1 = float(np.log(np.float32(1e-9), dtype=np.float32))  # -20.723267


def _split_excess_waits(nc, max_inline=1):
    """This walrus build accepts at most one sync-wait per instruction
    (f32 Matmult keeps LDWEIGHTS fused; STT/Drain structs too). Hoist all
    but one wait onto standalone same-engine EventSemaphore instructions."""
    import concourse.mybir as mybir

    n_split = 0
    for bb in nc.main_func.blocks:
        new_list = []
        changed = False
        for ins in bb.instructions:
            si = ins.sync_info
            if si is not None and si.on_wait and len(si.on_wait) > max_inline:
                waits = list(si.on_wait)
                hoistable = [w for w in waits if w.wait_reg is None]
                inline = [w for w in waits if w.wait_reg is not None]
                while hoistable and len(inline) < max_inline:
                    inline.append(hoistable.pop())
                for w in hoistable:
                    es = mybir.InstEventSemaphore(
                        name=f"I-waitsplit-{nc.next_id()}", ins=[], outs=[]
                    )
                    es.engine = ins.engine
                    es.sync_info = mybir.SyncInfo(
                        on_wait=[
                            mybir.SyncWait(
                                sync_type=w.sync_type,
                                id=w.id,
                                wait_mode=w.wait_mode,
                                ant_name=w.ant_name,
                                wait_value=w.wait_value,
                            )
                        ],
                        on_update=[],
                    )
                    new_list.append(es)
                    n_split += 1
                ins.sync_info = mybir.SyncInfo(
                    on_wait=inline, on_update=list(si.on_update)
                )
                changed = True
            new_list.append(ins)
        if changed:
            bb.instructions = new_list
    return n_split


def build_program():
    import concourse.bass as bass
    import concourse.tile as tile
    from concourse import mybir
    from concourse.masks import make_identity

    f32 = mybir.dt.float32
    f32r = mybir.dt.float32r
    i32 = mybir.dt.int32
    Alu = mybir.AluOpType
    Act = mybir.ActivationFunctionType

    nc = bass.Bass("TRN2", debug=False)

    q_d = nc.dram_tensor("pq", [C, LQ], f32, kind="ExternalInput").ap()
    k_d = nc.dram_tensor("pk", [C, LK], f32, kind="ExternalInput").ap()
    v_d = nc.dram_tensor("pv", [C, LK], f32, kind="ExternalInput").ap()
    m_d = nc.dram_tensor("pm", [LQ, LK], i32, kind="ExternalInput").ap()
    att_d = nc.dram_tensor("att", [LQ, LK], f32, kind="ExternalOutput").ap()
    out_d = nc.dram_tensor("out", [C, LQ], f32, kind="ExternalOutput").ap()

    with tile.TileContext(nc) as tc:
        with (
            tc.tile_pool(name="singles", bufs=1) as singles,
            tc.tile_pool(name="masks", bufs=3) as masks,
            tc.tile_pool(name="madds", bufs=3) as madds,
            tc.tile_pool(name="tsbs", bufs=3) as tsbs,
            tc.tile_pool(name="psbs", bufs=2) as psbs,
            tc.tile_pool(name="asbs", bufs=3) as asbs,
            tc.tile_pool(name="atsbs", bufs=2) as atsbs,
            tc.tile_pool(name="outs", bufs=2) as outs,
            tc.tile_pool(name="smalls", bufs=8) as smalls,
            tc.tile_pool(name="ps_e", bufs=2, space="PSUM") as ps_e,
            tc.tile_pool(name="ps_at", bufs=3, space="PSUM") as ps_at,
            tc.tile_pool(name="ps_pv", bufs=1, space="PSUM") as ps_pv,
        ):
            # ---- setup: resident tensors --------------------------------
            # f32r matmul operands must be produced "rounded": stage the f32
            # DMA then round via compute copy into f32r tiles.
            qsb = singles.tile([P, LQ], f32r)
            ksb = singles.tile([P, LK], f32r)
            vsb = singles.tile([P, LK], f32)
            stage_k = masks.tile([P, LK // 2], f32, tag="stage")
            nc.sync.dma_start(out=stage_k, in_=k_d[:, 0:LK // 2])
            nc.vector.tensor_copy(ksb[:, 0:LK // 2], stage_k)
            stage_q = masks.tile([P, LQ // 2], f32, tag="stage")
            nc.sync.dma_start(out=stage_q, in_=q_d[:, 0:LQ // 2])
            nc.vector.tensor_copy(qsb[:, 0:LQ // 2], stage_q)
            prefetched_masks = []
            mk = masks.tile([P, LK], i32, tag="mask")
            nc.sync.dma_start(out=mk, in_=m_d[0:P, :])
            prefetched_masks.append(mk)
            stage_k2 = masks.tile([P, LK // 2], f32, tag="stage")
            nc.sync.dma_start(out=stage_k2, in_=k_d[:, LK // 2:])
            nc.vector.tensor_copy(ksb[:, LK // 2:], stage_k2)
            mk1 = masks.tile([P, LK], i32, tag="mask")
            nc.sync.dma_start(out=mk1, in_=m_d[P:2 * P, :])
            prefetched_masks.append(mk1)
            stage_q2 = masks.tile([P, LQ // 2], f32, tag="stage")
            nc.sync.dma_start(out=stage_q2, in_=q_d[:, LQ // 2:])
            nc.vector.tensor_copy(qsb[:, LQ // 2:], stage_q2)
            nc.sync.dma_start(out=vsb, in_=v_d)

            ident = singles.tile([P, P], f32)
            make_identity(nc, ident)
            ident_r = singles.tile([P, P], f32r)
            nc.vector.tensor_copy(ident_r, ident)

            # V^T blocks: vt[:, kb, :] = V[:, kb*128:(kb+1)*128]^T
            vtsb = singles.tile([P, NKB, P], f32r)
            for g in range(NKB // 4):
                vt_ps = ps_at.tile([P, 4 * P], f32, tag="at_ps")
                for j in range(4):
                    kb = g * 4 + j
                    nc.tensor.transpose(
                        vt_ps[:, j * P:(j + 1) * P],
                        vsb[:, kb * P:(kb + 1) * P],
                        ident,
                    )
                nc.scalar.copy(
                    out=vtsb[:, g * 4:(g + 1) * 4, :],
                    in_=vt_ps.rearrange("p (a b) -> p a b", a=4),
                )

            # ---- main loop ----------------------------------------------
            for chunk in range(NTILES // CHUNK):
                atsb = atsbs.tile([P, NKB, CHUNK * P], f32r)
                for t in range(CHUNK):
                    qt = chunk * CHUNK + t
                    qs = qt * P

                    if qt < 2:
                        mask_t = prefetched_masks[qt]
                    else:
                        mask_t = masks.tile([P, LK], i32, tag="mask")
                        nc.sync.dma_start(out=mask_t, in_=m_d[qs:qs + P, :])

                    # madd = log(mask + 1e-9) exactly (0.0 or K1)
                    madd = madds.tile([P, LK], f32)
                    nc.gpsimd.tensor_scalar(
                        out=madd, in0=mask_t, scalar1=-K1, scalar2=K1,
                        op0=Alu.mult, op1=Alu.add,
                    )

                    # e = Q_tile^T @ K ; t = e*scale + madd  (1024-col halves)
                    t_sb = tsbs.tile([P, LK], f32)
                    for h in range(2):
                        e_ps = ps_e.tile([P, 1024], f32, tag="e_ps")
                        for n in range(2):
                            col = h * 1024 + n * 512
                            nc.tensor.matmul(
                                e_ps[:, n * 512:(n + 1) * 512],
                                qsb[:, qs:qs + P],
                                ksb[:, col:col + 512],
                                start=True, stop=True,
                            )
                        nc.vector.scalar_tensor_tensor(
                            out=t_sb[:, h * 1024:(h + 1) * 1024],
                            in0=e_ps, scalar=SCALE,
                            in1=madd[:, h * 1024:(h + 1) * 1024],
                            op0=Alu.mult, op1=Alu.add,
                        )

                    # p = exp(t), D = rowsum(p)
                    p_sb = psbs.tile([P, LK], f32)
                    d_sb = smalls.tile([P, 1], f32, tag="d")
                    nc.scalar.activation(
                        out=p_sb, in_=t_sb, func=Act.Exp,
                        bias=0.0, scale=1.0, accum_out=d_sb,
                    )
                    r_sb = smalls.tile([P, 1], f32, tag="r")
                    nc.vector.reciprocal(out=r_sb, in_=d_sb)

                    # A = p * (1/D)
                    a_sb = asbs.tile([P, LK], f32r)
                    nc.vector.tensor_scalar_mul(a_sb, p_sb, r_sb)
                    (nc.scalar if qt % 2 == 0 else nc.sync).dma_start(out=att_d[qs:qs + P, :], in_=a_sb.bitcast(f32))

                    # A^T blocks -> atsb[:, kb, t*128:(t+1)*128]
                    for g in range(NKB // 4):
                        at_ps = ps_at.tile([P, 4 * P], f32, tag="at_ps")
                        for j in range(4):
                            kb = g * 4 + j
                            nc.tensor.transpose(
                                at_ps[:, j * P:(j + 1) * P].bitcast(f32r),
                                a_sb[:, kb * P:(kb + 1) * P],
                                ident_r,
                            )
                        if g == 3:
                            nc.vector.tensor_copy(
                                atsb[:, g * 4:(g + 1) * 4, t * P:(t + 1) * P],
                                at_ps.rearrange("p (a b) -> p a b", a=4),
                            )
                        else:
                            nc.scalar.copy(
                                out=atsb[:, g * 4:(g + 1) * 4, t * P:(t + 1) * P],
                                in_=at_ps.rearrange("p (a b) -> p a b", a=4),
                            )

                # out chunk: sum over k-blocks of V^T_kb @ A^T[kb]
                pv_ps = ps_pv.tile([P, CHUNK * P], f32)
                for kb in range(NKB):
                    nc.tensor.matmul(
                        pv_ps, vtsb[:, kb, :],
                        atsb[:, kb, :],
                        start=(kb == 0), stop=(kb == NKB - 1),
                    )
                o_sb = outs.tile([P, CHUNK * P], f32)
                nc.scalar.copy(out=o_sb, in_=pv_ps)
                nc.scalar.dma_start(
                    out=out_d[:, chunk * CHUNK * P:(chunk + 1) * CHUNK * P],
                    in_=o_sb,
                )

    _split_excess_waits(nc)
    return nc


_NC = None


def _get_nc():
    global _NC
    if _NC is None:
        _NC = build_program()
    return _NC


def make_in_maps(proj_query, proj_key, proj_val, padding_mask):
    in_maps = []
    for b in range(B):
        in_maps.append({
            "pq": np.ascontiguousarray(proj_query[b], dtype=np.float32),
            "pk": np.ascontiguousarray(proj_key[b], dtype=np.float32),
            "pv": np.ascontiguousarray(proj_val[b], dtype=np.float32),
            "pm": np.ascontiguousarray(padding_mask[b], dtype=np.int32),
        })
    return in_maps


def kernel(proj_query, proj_key, proj_val, padding_mask):
    from concourse.bass_utils import run_bass_kernel_spmd

    nc = _get_nc()
    in_maps = make_in_maps(proj_query, proj_key, proj_val, padding_mask)
    res = run_bass_kernel_spmd(nc, in_maps, core_ids=list(range(B)))
    out = np.stack([res.results[b]["out"] for b in range(B)])
    att = np.stack([res.results[b]["att"] for b in range(B)])
    return out, att
